# revision 21
# baseline (speedup 1.0000x reference)
"""KNN k-th-nearest-distance kernel for Trainium2 (8 NeuronCores).

Math: reference computes, per query row n, the k-th smallest of
dist[n,m] = sqrt(max(||zn||^2 + ||rn||^2 - 2 zn.rn, 1e-12)) over m,
with zn/rn the L2-normalized rows of z [2048,128] / ref [50000,128].
Since ||zn||^2 == ||rn||^2 == 1 (to fp32 rounding), dist is a
monotonically *decreasing* function of s = zn.rn, so the k-th smallest
distance corresponds to the k-th *largest* inner product s.

Device work (ref sharded across the 8 cores, queries replicated):
  Phase A (per core, own 1/8 ref slice, zero-padded to 6400 rows):
    normalize rows in fp32 (square-accum + scale on ScalarE,
    reciprocal on VectorE), cast bf16, PE-transpose -> rnT slice
    [128, 6400] resident in SBUF. 1/norm = 1/max(sqrt(ssq), 1e-20) so
    padded zero rows give sims of exactly 0 (never competitive: real
    top-11 sims are ~ +0.37|z| for randn data).
  Phase B (software-pipelined with A per 512/1024-col piece): for each
    of the 16 query blocks, s_raw = z_blk @ rnT_piece via bf16 PE
    matmul into PSUM windows; DVE max8 per window -> top-8 window
    candidates; per-block max8 + match_replace8 + max8 merge -> this
    core's local top-16 per query, DMA'd to the output.
  The global 8-way merge of per-core top-16s happens on host (128
    candidate values per query contain the exact global top-16).
Host: divide by |z_n|, dist = sqrt(max(2 - 2*s, 1e-12)), pick column k.
"""

import os
import sys

sys.path.insert(0, "/opt/trn_rl_repo")

from contextlib import ExitStack

import numpy as np


def _install_ntff_hook_shim():
    """The agent image's antenv lacks axon_hooks, so trace=True degrades.
    Recreate the hook module + the ctypes NTFF driver (mirrors
    trn_agent_boot.trn_boot._ntff_profile_via_ctypes)."""
    import contextlib
    import ctypes
    import types

    if "antenv.axon_hooks" in sys.modules:
        return
    so_path = "/opt/axon/libaxon_pjrt.so"
    mod = types.ModuleType("antenv.axon_hooks")
    state = {"hook": None}

    def set_axon_ntff_profile_hook(h):
        state["hook"] = h

    def get_axon_ntff_profile_hook():
        return state["hook"]

    mod.set_axon_ntff_profile_hook = set_axon_ntff_profile_hook
    mod.get_axon_ntff_profile_hook = get_axon_ntff_profile_hook
    sys.modules["antenv.axon_hooks"] = mod

    try:
        lib = ctypes.CDLL(so_path)
        if not hasattr(lib, "axon_start_nrt_profile"):
            return
        lib.axon_start_nrt_profile.argtypes = [
            ctypes.POINTER(ctypes.c_int64),
            ctypes.c_size_t,
        ]
        lib.axon_start_nrt_profile.restype = ctypes.c_int64
        lib.axon_stop_nrt_profile.argtypes = [ctypes.c_char_p]
        lib.axon_stop_nrt_profile.restype = ctypes.c_int64

        @contextlib.contextmanager
        def _hook(output_dir, device_ids):
            import jax

            jax.devices()
            if device_ids:
                ids = (ctypes.c_int64 * len(device_ids))(*device_ids)
                rc = lib.axon_start_nrt_profile(ids, len(device_ids))
            else:
                rc = lib.axon_start_nrt_profile(None, 0)
            if rc != 0:
                raise RuntimeError(f"axon_start_nrt_profile rc={rc}")
            try:
                yield
            finally:
                n = lib.axon_stop_nrt_profile(str(output_dir).encode())
                print(f"ntff profile: {n} file(s) written to {output_dir}")

        state["hook"] = _hook
    except Exception:
        pass


_install_ntff_hook_shim()

import concourse.bacc as bacc
import concourse.bass as bass
import concourse.tile as tile
from concourse import mybir
from concourse.bass_utils import run_bass_kernel_spmd

N, M, D = 2048, 50000, 128
NCORES = 8
NBLK = N // 128  # 16 query blocks of 128
SLICE = 6400  # ref rows prepped per core (core 7 zero-padded)
WINS = (512, 512, 1024, 1024, 1024, 1024, 1024, 256)  # max8 windows over slice cols
NWIN = len(WINS)
CANDW = NWIN * 8  # per-core candidate width per query row (32)

F32 = mybir.dt.float32
BF16 = mybir.dt.bfloat16

_CACHE = {}
LAST_RESULTS = None


def _build():
    nc = bacc.Bacc(
        "TRN2", target_bir_lowering=False, debug=False, num_devices=NCORES
    )
    z_d = nc.dram_tensor("z", [N, D], F32, kind="ExternalInput")
    ref_d = nc.dram_tensor("refsl", [SLICE, D], F32, kind="ExternalInput")
    out_d = nc.dram_tensor("top16", [N, 16], F32, kind="ExternalOutput")
    import ml_dtypes

    ident_d = nc.inline_tensor(
        np.eye(128, dtype=np.float32).astype(ml_dtypes.bfloat16), name="ident"
    )

    z = z_d.ap()
    ref = ref_d.ap()
    out16 = out_d.ap()

    with tile.TileContext(nc) as tc, ExitStack() as ctx:
        const_pool = ctx.enter_context(tc.tile_pool(name="const", bufs=1))
        persist = ctx.enter_context(tc.tile_pool(name="persist", bufs=1))
        cand_pool = ctx.enter_context(tc.tile_pool(name="cand", bufs=1))
        fin_pool = ctx.enter_context(tc.tile_pool(name="fin", bufs=2))

        ident = const_pool.tile([128, 128], BF16, name="ident")
        nc.sync.dma_start(ident[:, :], ident_d.ap()[:, :])

        rnT = [
            persist.tile([128, w], BF16, tag=f"rnT{i}", name=f"rnT{i}")
            for i, w in enumerate(WINS)
        ]
        znT = [
            persist.tile([128, 128], BF16, tag=f"znT{b}", name=f"znT{b}")
            for b in range(NBLK)
        ]
        cand = [
            cand_pool.tile([128, CANDW], F32, tag=f"cand{b}", name=f"cand{b}")
            for b in range(NBLK)
        ]

        # ---- Phase A: z transposes + own-slice normalize/transpose ----
        rload_pool = ctx.enter_context(tc.tile_pool(name="rload", bufs=3))
        sq_pool = ctx.enter_context(tc.tile_pool(name="sq", bufs=2))
        stat_pool = ctx.enter_context(tc.tile_pool(name="stat", bufs=3))
        rsc_pool = ctx.enter_context(tc.tile_pool(name="rsc", bufs=3))
        tpsum_pool = ctx.enter_context(tc.tile_pool(name="tps", bufs=2, space="PSUM"))
        zl_pool = ctx.enter_context(tc.tile_pool(name="zl", bufs=3))
        spsum_pool = ctx.enter_context(tc.tile_pool(name="sp", bufs=3, space="PSUM"))
        if True:
            for b in range(NBLK):
                zt = zl_pool.tile([128, D], F32, tag="zload")
                nc.sync.dma_start(zt[:, :], z[b * 128 : (b + 1) * 128, :])
                ztb = zl_pool.tile([128, D], BF16, tag="zloadb")
                nc.scalar.copy(ztb[:, :], zt[:, :])
                zp = tpsum_pool.tile([128, 512], BF16, tag="tps")
                nc.tensor.transpose(zp[:, :128], ztb[:, :], ident[:, :])
                nc.scalar.copy(znT[b][:, :], zp[:, :128])

            pass

        # ---- piece-pipelined: A-prep for piece p, then B matmul+max8
        # over piece p for all 16 query blocks ----
        col = 0
        for p_i, pw in enumerate(WINS):
            # phase A for this piece
            pcol = 0
            while pcol < pw:
                gw = min(512, pw - pcol)
                G = gw // 128
                rl = rload_pool.tile([128, 4, 128], F32, tag="rload")
                nc.sync.dma_start(
                    rl[:, :G, :],
                    ref[col : col + gw, :].rearrange("(g p) d -> p g d", p=128),
                )
                ssq = stat_pool.tile([128, 4], F32, tag="ssq")
                for g in range(G):
                    sq = sq_pool.tile([128, 128], F32, tag="sq")
                    nc.scalar.activation(
                        sq[:, :],
                        rl[:, g, :],
                        mybir.ActivationFunctionType.Square,
                        accum_out=ssq[:, g : g + 1],
                    )
                sn = stat_pool.tile([128, 4], F32, tag="sn")
                nc.scalar.sqrt(sn[:, :G], ssq[:, :G])
                # clamp so zero-padded rows give 0*big = 0, not NaN
                sc = stat_pool.tile([128, 4], F32, tag="sc")
                nc.vector.tensor_scalar_max(sc[:, :G], sn[:, :G], 1e-20)
                rq = stat_pool.tile([128, 4], F32, tag="rq")
                nc.vector.reciprocal(rq[:, :G], sc[:, :G])

                rsc = rsc_pool.tile([128, 4, 128], BF16, tag="rsc")
                for g in range(G):
                    nc.scalar.mul(rsc[:, g, :], rl[:, g, :], rq[:, g : g + 1])
                tp = tpsum_pool.tile([128, 512], BF16, tag="tps")
                for g in range(G):
                    nc.tensor.transpose(
                        tp[:, g * 128 : (g + 1) * 128],
                        rsc[:, g, :],
                        ident[:, :],
                    )
                nc.scalar.copy(rnT[p_i][:, pcol : pcol + gw], tp[:, :gw])
                pcol += gw
                col += gw
            # phase B for this piece (local top-16 merge fused into the
            # last piece; global 8-way merge happens on host)
            last = p_i == len(WINS) - 1
            for b in range(NBLK):
                sp = spsum_pool.tile([128, 1024], F32, tag="sp")
                for h in range(0, pw, 512):
                    hw = min(512, pw - h)
                    nc.tensor.matmul(
                        sp[:, h : h + hw],
                        znT[b][:, :],
                        rnT[p_i][:, h : h + hw],
                        start=True,
                        stop=True,
                    )
                nc.vector.max(
                    cand[b][:, p_i * 8 : (p_i + 1) * 8], sp[:, :pw]
                )
                if last:
                    t16 = fin_pool.tile([128, 16], F32, tag="t16")
                    nc.vector.max(t16[:, 0:8], cand[b][:, :])
                    cand2 = fin_pool.tile([128, CANDW], F32, tag="cand2")
                    nc.vector.match_replace(
                        cand2[:, :], t16[:, 0:8], cand[b][:, :], -3.0
                    )
                    nc.vector.max(t16[:, 8:16], cand2[:, :])
                    nc.sync.dma_start(
                        out16[b * 128 : (b + 1) * 128, :], t16[:, :]
                    )

    nc.compile()
    return nc


def kernel(z, ref, k):
    global LAST_RESULTS
    z_np = np.ascontiguousarray(np.asarray(z, dtype=np.float32))
    ref_np = np.ascontiguousarray(np.asarray(ref, dtype=np.float32))
    kk = int(k)

    if "nc" not in _CACHE:
        _CACHE["nc"] = _build()
    nc = _CACHE["nc"]

    refp = np.zeros((NCORES * SLICE, D), dtype=np.float32)
    refp[:M] = ref_np
    in_maps = [
        {
            "z": z_np,
            "refsl": np.ascontiguousarray(refp[i * SLICE : (i + 1) * SLICE]),
        }
        for i in range(NCORES)
    ]
    res = run_bass_kernel_spmd(nc, in_maps, core_ids=list(range(NCORES)))
    LAST_RESULTS = res
    # each core returns its slice-local top-16 per query; merge on host
    allc = np.concatenate(
        [r["top16"] for r in res.results], axis=1
    )  # [N, 128]
    allc.sort(axis=1)
    s_k = allc[:, -(kk + 1)]  # (k+1)-th largest raw score

    znorm = np.sqrt(np.sum(z_np.astype(np.float32) ** 2, axis=1))  # [N]
    s = s_k / znorm
    return np.sqrt(np.maximum(2.0 - 2.0 * s, 1e-12)).astype(np.float32)
